# revision 30
# baseline (speedup 1.0000x reference)
"""Trainium2 Bass kernel for nn_ExactAttention (block-diagonal sparse attention).

Reference computes dense softmax attention over [N,N] then masks to
block-diagonal segments (batch_seg is sorted).  Only the diagonal blocks
survive, so we compute segment-local attention only.

The reference subtracts the *global* max of Q@K^T before exp; softmax is
shift-invariant except through EPS=1e-8, whose effect is ~1e-8 relative,
so we skip the max entirely (max |dot|/sqrt(d) ~ 6 -> exp ~ 400, no
overflow even in fp16).

Sharding: segments are sorted by length (desc) and dealt round-robin:
slot j of every core gets one of ranks [8j, 8j+8), all padded to the
group max L_j, so all 8 cores run one SPMD program with near-zero
padding waste and balanced work.  Largest slot first (warm-up junk
matmuls cover the cold-clock phase), smallest last (shortest tail).

Precision/perf choices (rel-err budget is 2e-2; this lands ~5e-4):
  * scores: ONE fp16 matmul per key chunk (full PE rate), fp32 PSUM.
  * exp on ACT reads chunk-PAIRS of PSUM banks in one strided
    instruction (halves the ~250ns/instr overhead), writes P as fp16.
  * AV in fp16, V-stationary: O^T [128 x qbs] += V_c^T P_c.
  * denominator on PE: ones-vector matmuls accumulate sum_k P[k,q] into
    partition rows 32*(r//2) of two dedicated PSUM banks (col-group
    granularity); each bank is gathered+shipped as soon as its last
    slot is done (full-tile DVE read avoids PE-W/DVE-R bank races).
    Padded key rows (zero K) give exp(0)=1; the host subtracts (L-len).
  * O ships as fp16; host divides by den.
  * software pipeline: scores/exp of slot s+1 issue before AV/den of
    slot s, so the PE never idles waiting on the ACT exp.
  * all input DMAs issue upfront; PE HAM warm-up bridges the first DMA
    wait; a dummy exp forces the ACT table load off the critical path.
"""

import numpy as np

import concourse.bass as bass
import concourse.mybir as mybir
import concourse.tile as tile
from concourse import bacc
from concourse import bass_utils

D = 128
N_CORES = 8
EPS = 1e-8
F32 = mybir.dt.float32
F16 = mybir.dt.float16

_program_cache = {}


def _qblocks(L, split=False):
    """Query blocks of <=512 (split mode retired: the guaranteed-warm
    start makes sub-blocking the first slot a net loss)."""
    return [(qb0, min(512, L - qb0)) for qb0 in range(0, L, 512)]


def _build_program(slot_lens):
    """Build + compile the SPMD program for per-slot padded lengths."""
    key = tuple(slot_lens)
    if key in _program_cache:
        return _program_cache[key]

    scale = float(1.0 / np.sqrt(np.float32(D)))
    R = sum(slot_lens)
    offs = np.concatenate([[0], np.cumsum(slot_lens)]).astype(int)
    nkcs = [(L + 127) // 128 for L in slot_lens]
    choffs = np.concatenate([[0], np.cumsum(nkcs)]).astype(int)
    C = int(choffs[-1])

    # den rows: one per (slot, qblock); row r -> bank r%2, partition
    # 32*(r//2) (PE col-group granularity is 32)
    n_dr = sum(len(_qblocks(L, s == 0)) for s, L in enumerate(slot_lens))
    assert n_dr <= 8, "too many (slot, qblock) pairs for den banks"

    nc = bacc.Bacc("TRN2", target_bir_lowering=False, debug=False,
                   num_devices=N_CORES)

    # packed [q | k] per slot (fp16): big contiguous per-partition runs
    qk_d = nc.dram_tensor("qk", [D, 2 * R], F16, kind="ExternalInput").ap()
    vx_d = nc.dram_tensor("vx", [D, C * 128], F16, kind="ExternalInput").ap()
    o_d = nc.dram_tensor("o", [D, R], F16, kind="ExternalOutput").ap()
    den_d = nc.dram_tensor("den", [8, 512], F32, kind="ExternalOutput").ap()

    n_slots = len(slot_lens)
    # last slot whose den rows land in each bank (gather as soon as done)
    last_bank_slot = {}
    dr = 0
    for s, L in enumerate(slot_lens):
        for _ in _qblocks(L, s == 0):
            last_bank_slot[dr % 2] = s
            dr += 1

    with tile.TileContext(nc) as tc:
        with tc.tile_pool(name="qk", bufs=n_slots) as qk_pool, \
             tc.tile_pool(name="v", bufs=n_slots) as v_pool, \
             tc.tile_pool(name="p", bufs=4) as p_pool, \
             tc.tile_pool(name="osb", bufs=3) as o_pool, \
             tc.tile_pool(name="tps", bufs=2, space="PSUM") as t_psum, \
             tc.tile_pool(name="ops", bufs=2, space="PSUM") as o_psum, \
             tc.tile_pool(name="dps", bufs=1, space="PSUM") as d_psum:

            # junk operand for PE warm-up + ones column for den matmuls;
            # memset these FIRST so the warm-up chain starts ASAP
            warm_sb = p_pool.tile([128, 128], F16, tag="warm", bufs=1)
            nc.vector.memset(warm_sb[:], 0.0)
            ones_sb = p_pool.tile([128, 8], F16, tag="ones", bufs=1)
            nc.vector.memset(ones_sb[:], 1.0)

            # All input DMAs issued upfront, ordered by need across the
            # two HWDGE rings so nothing steals bandwidth from the
            # earliest-needed transfers.  dma_start desc-gen costs the
            # issuing engine ~650ns, so the scalar engine only issues
            # DMAs BEFORE its first real exp (~10.9us); gpsimd stays
            # silent (an early SWDGE desc would also expand the measured
            # window backwards).
            qk_sbs, v_sbs = [], []
            for s, L in enumerate(slot_lens):
                qk_sb = qk_pool.tile([D, 2 * L], F16, tag="qk",
                                     name=f"qk{s}")
                vs = v_pool.tile([D, nkcs[s] * 128], F16, tag="v",
                                 name=f"v{s}")
                qk_sbs.append(qk_sb)
                v_sbs.append(vs)

            def v_src(s):
                c0 = int(choffs[s])
                return vx_d[:, c0 * 128:(c0 + nkcs[s]) * 128]

            # slot 0 quartered: Q halves on sync, K halves on scalar, so
            # its first q-block can start on ~60KB of landed input
            L0 = slot_lens[0]
            m0 = (L0 + 1) // 2
            k0b = min(2 * 128, L0)  # first K chunk-pair
            nc.scalar.dma_start(qk_sbs[0][:, L0:L0 + k0b],
                                qk_d[:, L0:L0 + k0b])
            if k0b < L0:
                nc.scalar.dma_start(qk_sbs[0][:, L0 + k0b:],
                                    qk_d[:, L0 + k0b:2 * L0])
            nc.sync.dma_start(qk_sbs[0][:, :m0], qk_d[:, :m0])
            nc.sync.dma_start(qk_sbs[0][:, m0:L0], qk_d[:, m0:L0])
            # qk1 before V0: it is needed first and V0 was delaying it
            for s in range(1, n_slots):
                o0 = int(offs[s])
                L = slot_lens[s]
                nc.sync.dma_start(qk_sbs[s][:],
                                  qk_d[:, 2 * o0:2 * (o0 + L)])
                if s == 1:
                    nc.sync.dma_start(v_sbs[0][:], v_src(0))
            for s in range(1, n_slots):
                nc.sync.dma_start(v_sbs[s][:], v_src(s))

            # two dedicated den banks, alive for the whole program
            den_a = d_psum.tile([128, 512], F32, tag="dena")
            den_b = d_psum.tile([128, 512], F32, tag="denb")
            den_banks = [den_a, den_b]
            nc.vector.memset(den_a[:], 0.0)
            nc.vector.memset(den_b[:], 0.0)

            # dummy exp pulls the ~2.7us ACT table load off the critical
            # path (overlaps the first DMA wait)
            wexp = p_pool.tile([128, 8], F16, tag="wexp", bufs=1)
            nc.scalar.activation(wexp[:, :1], ones_sb[:, :1],
                                 mybir.ActivationFunctionType.Exp, scale=1.0)

            # PE warm-up: HAM releases the clock throttle after ~3.4us of
            # sustained PE activity; junk fp16 matmuls bridge the initial
            # DMA wait (sized to end roughly when the first inputs land)
            warm_ps = o_psum.tile([128, 512], F32, tag="ops")
            for _ in range(26):
                nc.tensor.matmul(warm_ps[:8, :128], warm_sb[:, :8],
                                 warm_sb[:, :], start=True, stop=True)

            def scores_stage(s):
                """Scores matmuls + exp for slot s; returns P tiles."""
                L = slot_lens[s]
                qk_sb = qk_sbs[s]
                qhs = qk_sb[:, 0:L]
                khs = qk_sb[:, L:2 * L]
                nkc = nkcs[s]
                out = []
                for qb0, qbs in _qblocks(L, s == 0):
                    qslc = slice(qb0, qb0 + qbs)
                    # chunk-pairs: [128, 2, 512] = 2 PSUM banks, exp'd in
                    # one strided ACT instruction per pair
                    p_tiles = []
                    for pj in range(0, nkc, 2):
                        npair = min(2, nkc - pj)
                        t_ps = t_psum.tile([128, 2, 512], F32, tag="t",
                                           name=f"t{s}_{qb0}_{pj}")
                        for j in range(npair):
                            c = pj + j
                            ck = min(128, L - c * 128)
                            kslc = slice(c * 128, c * 128 + ck)
                            nc.tensor.matmul(
                                t_ps[:ck, j:j + 1, :qbs].squeeze(1),
                                khs[:, kslc], qhs[:, qslc],
                                start=True, stop=True)
                        p_sb = p_pool.tile([128, 2 * 512], F16, tag="p",
                                           name=f"p{s}_{qb0}_{pj}")
                        nc.scalar.activation(
                            p_sb[:, :npair * qbs].rearrange(
                                "p (c q) -> p c q", c=npair),
                            t_ps[:, :npair, :qbs],
                            mybir.ActivationFunctionType.Exp, scale=scale)
                        p_tiles.append(p_sb)
                    out.append((qb0, qbs, p_tiles))
                return out

            dr = 0

            def finish_stage(s, sc):
                """AV + den matmuls, O copy + store for slot s."""
                nonlocal dr
                L = slot_lens[s]
                nkc = nkcs[s]
                o0 = int(offs[s])
                vs = v_sbs[s]
                for qb0, qbs, p_tiles in sc:
                    def p_ap(c, ck):
                        return p_tiles[c // 2][:ck, (c % 2) * qbs:
                                               (c % 2 + 1) * qbs]

                    # AV: O^T += V_c^T P_c (V stationary, one weight/chunk)
                    o_ps = o_psum.tile([128, qbs], F32, tag="ops",
                                       name=f"ops{s}_{qb0}")
                    for c in range(nkc):
                        ck = min(128, L - c * 128)
                        nc.tensor.matmul(o_ps[:],
                                         vs[:ck, c * 128:(c + 1) * 128],
                                         p_ap(c, ck),
                                         start=(c == 0), stop=(c == nkc - 1))

                    # den: ones^T P_c accumulated at partition 32*(dr//2)
                    # of bank dr%2
                    dt_ = den_banks[dr % 2]
                    dp0 = 32 * (dr // 2)
                    for c in range(nkc):
                        ck = min(128, L - c * 128)
                        nc.tensor.matmul(dt_[dp0:dp0 + 1, :qbs],
                                         ones_sb[:ck, :1], p_ap(c, ck),
                                         start=(c == 0), stop=(c == nkc - 1),
                                         tile_position=(0, dp0))
                    dr += 1

                    # O copy + store; mid slots ship halves pipelined, the
                    # last slot ships ONE dma (one less serial ~600ns
                    # desc-gen in the tail)
                    os_sb = o_pool.tile([128, qbs], F16, tag="o",
                                        name=f"o{s}_{qb0}")
                    h = qbs // 2
                    d0 = o0 + qb0
                    nc.vector.tensor_copy(os_sb[:, :h], o_ps[:, :h])
                    if s == n_slots - 1:
                        nc.vector.tensor_copy(os_sb[:, h:], o_ps[:, h:])
                        nc.sync.dma_start(o_d[:, d0:d0 + qbs], os_sb[:])
                    else:
                        nc.sync.dma_start(o_d[:, d0:d0 + h], os_sb[:, :h])
                        nc.vector.tensor_copy(os_sb[:, h:], o_ps[:, h:])
                        nc.sync.dma_start(o_d[:, d0 + h:d0 + qbs],
                                          os_sb[:, h:])

                # den bank done forever? gather + ship it now (full-tile
                # DVE read depends on every writer: no PE-W/DVE-R race)
                for b in range(2):
                    if last_bank_slot.get(b) == s:
                        nrow = (n_dr + 1 - b) // 2
                        w = max(qbs_ for r_, qbs_ in enumerate(
                            q for s_, L_ in enumerate(slot_lens)
                            for _, q in _qblocks(L_, s_ == 0))
                            if r_ % 2 == b)
                        den_sb = o_pool.tile([128, 512], F32, tag=f"den{b}",
                                             bufs=1, name=f"densb{b}")
                        # the final bank's gather runs on the scalar
                        # engine (idle after the last exp) so it overlaps
                        # the DVE's O copies in the tail
                        if s == n_slots - 1:
                            nc.scalar.copy(den_sb[:, :w],
                                           den_banks[b][:, :w])
                            nc.scalar.dma_start(
                                den_d[4 * b:4 * b + nrow, :w],
                                den_sb[0:32 * nrow:32, :w])
                        else:
                            nc.vector.tensor_copy(den_sb[:, :w],
                                                  den_banks[b][:, :w])
                            nc.sync.dma_start(
                                den_d[4 * b:4 * b + nrow, :w],
                                den_sb[0:32 * nrow:32, :w])

            # software pipeline: scores/exp of slot s+1 are issued before
            # AV/den of slot s so the PE never idles waiting on exp
            prev = None
            for s in range(n_slots):
                sc = scores_stage(s)
                if prev is not None:
                    finish_stage(prev[0], prev[1])
                prev = (s, sc)
            finish_stage(prev[0], prev[1])

    nc.compile()
    _program_cache[key] = nc
    return nc


def _reference_host(Q, K, V, num_batch, batch_seg):
    """Pure-NumPy fallback for input shapes outside the tuned envelope."""
    dot = Q.astype(np.float64) @ K.T.astype(np.float64)
    A = np.exp((dot - dot.max()) / np.sqrt(np.float64(Q.shape[-1])))
    if num_batch > 1:
        A = np.where(batch_seg[None, :] == batch_seg[:, None], A, 0.0)
    return ((A / (A.sum(-1, keepdims=True) + EPS)) @ V.astype(np.float64)
            ).astype(np.float32)


def kernel(Q, K, V, num_batch, batch_seg):
    Q = np.asarray(Q, dtype=np.float32)
    K = np.asarray(K, dtype=np.float32)
    V = np.asarray(V, dtype=np.float32)
    batch_seg = np.asarray(batch_seg)
    N = Q.shape[0]
    nb = int(num_batch)

    counts = np.bincount(batch_seg.astype(np.int64), minlength=max(nb, 1))
    if nb < 2 or (counts.max() if nb else N) > 1024:
        return _reference_host(Q, K, V, nb, batch_seg)

    # row indices per segment (robust to unsorted batch_seg)
    row_order = np.argsort(batch_seg, kind="stable")
    starts = np.zeros(nb + 1, dtype=np.int64)
    np.cumsum(counts, out=starts[1:])

    # rank segments by length desc, group into slots of 8, largest slot
    # first (it overlaps the warm-up), smallest last (shortest tail)
    order = np.argsort(-counts, kind="stable")
    n_slots = (nb + N_CORES - 1) // N_CORES
    slot_lens = []
    assign = {}  # (core, slot) -> seg id
    for j in range(n_slots):
        grp = order[j * N_CORES:(j + 1) * N_CORES]
        slot_lens.append(max(1, int(counts[grp].max())))
        for c, seg in enumerate(grp):
            assign[(c, j)] = int(seg)

    # den row capacity: 8 (slot, qblock) pairs
    if sum(len(_qblocks(L, j == 0)) for j, L in enumerate(slot_lens)) > 8:
        return _reference_host(Q, K, V, nb, batch_seg)

    offs = np.concatenate([[0], np.cumsum(slot_lens)]).astype(int)
    nkcs = [(L + 127) // 128 for L in slot_lens]
    choffs = np.concatenate([[0], np.cumsum(nkcs)]).astype(int)
    R = int(offs[-1])
    C = int(choffs[-1])

    nc = _build_program(tuple(slot_lens))

    in_maps = []
    for core in range(N_CORES):
        Qp = np.zeros((R, D), np.float32)
        Kp = np.zeros((R, D), np.float32)
        Vp = np.zeros((C * 128, D), np.float32)
        for j in range(n_slots):
            seg = assign.get((core, j))
            if seg is None:
                continue
            b0, b1 = starts[seg], starts[seg + 1]
            ln = int(b1 - b0)
            if ln == 0:
                continue
            ridx = row_order[b0:b1]
            o0 = int(offs[j])
            Qp[o0:o0 + ln] = Q[ridx]
            Kp[o0:o0 + ln] = K[ridx]
            v0 = int(choffs[j]) * 128
            Vp[v0:v0 + ln] = V[ridx]
        qt = Qp.T.astype(np.float16)
        kt = Kp.T.astype(np.float16)
        qk = np.empty((D, 2 * R), np.float16)
        for j in range(n_slots):
            o0, L = int(offs[j]), slot_lens[j]
            qk[:, 2 * o0:2 * o0 + L] = qt[:, o0:o0 + L]
            qk[:, 2 * o0 + L:2 * (o0 + L)] = kt[:, o0:o0 + L]
        vh = np.ascontiguousarray(
            Vp.astype(np.float16).reshape(C, 128, D).transpose(1, 0, 2)
        ).reshape(D, C * 128)
        in_maps.append({"qk": qk, "vx": vh})

    global _last_in_maps
    _last_in_maps = in_maps
    res = bass_utils.run_bass_kernel_spmd(nc, in_maps,
                                          core_ids=list(range(N_CORES)))

    # den row index per (slot, qblock)
    dr_of = {}
    dr = 0
    for j, L in enumerate(slot_lens):
        for qb0, qbs in _qblocks(L, j == 0):
            dr_of[(j, qb0)] = dr
            dr += 1

    out = np.empty((N, D), np.float32)
    for (core, j), seg in assign.items():
        b0, b1 = starts[seg], starts[seg + 1]
        ln = int(b1 - b0)
        if ln == 0:
            continue
        o0 = int(offs[j])
        L = slot_lens[j]
        ot = res.results[core]["o"].astype(np.float32)    # [D, R]
        dn = res.results[core]["den"]                     # [8, 512] f32
        den = np.empty(L, np.float64)
        for qb0, qbs in _qblocks(L, j == 0):
            r = dr_of[(j, qb0)]
            den[qb0:qb0 + qbs] = dn[4 * (r % 2) + r // 2, :qbs]
        den = den[:ln] - float(L - ln) + EPS
        out[row_order[b0:b1]] = (ot[:, o0:o0 + ln].T / den[:, None]
                                 ).astype(np.float32)
    return out
